# revision 16
# baseline (speedup 1.0000x reference)
"""MoE ConditionalFeedForward (SwiGLU top-2 of 8 experts) on 8 Trainium2 cores.

Strategy: expert-parallel. Core c owns expert c's weights. The host routes
tokens: (token, expert) assignments are DEDUPED (a token listing the same
expert in both top-2 slots is computed once) and bucketed by expert; each
core runs the dense SwiGLU FFN for up to C<=512 of its expert's unique
tokens. Assignments beyond 512 per expert (spill; empty for the reference
distribution after dedup) are computed on the host.

Everything on-device runs in bfloat16 (tolerance is 2e-2; bf16 end-to-end
measures ~5e-3 max-rel). bf16 halves HBM traffic vs fp32r, enables the
fast-weight-load path so LDWEIGHTS fully hides under matmuls, and halves
SBUF pressure. Matmul accumulation is fp32 in PSUM.

Layouts are feature-major ("transposed") end to end so the contraction dim
always sits on SBUF partitions and no on-device transposes are needed:
  phase 1: h1T/h3T[i, t] = sum_d w1T[d, i] * xT[d, t]   (lhsT=w1 chunk, rhs=x)
  fuse:    hT = silu(h1T) * h3T                          (bf16, per-ic tiles)
  phase 2: outT[d, t]    = sum_i w2T[i, d] * hT[i, t]

hT is 32 separate per-ic tiles so phase-2 matmuls depend only on the ic
they read; ps1/ps2 PSUM pools are both open (disjoint banks) so phase 2
can start while phase 1 drains. Phase-2 accumulation alternates between
two PSUM banks (kic parity, merged by a DVE copy+add). The last d-chunk
is split [C-128, 128] so the final copy+DMA tail is short.

DMA model (measured): a queue sustains up to ~340 GB/s, but each engine
dispatches only ~1 descriptor per 0.65us and the fabric ramps over the
first ~10us — so the early window wants FEW, BIG transfers spread across
the three dispatching engines (sync, gpsimd, scalar). Weights stream as
full 1MB per-ic tiles (w1 on sync, w3 on gpsimd, ~147 GB/s each, 2.3x
headroom); ic0's tiles are split in halves and x's four 0.5MB chunks are
placed so every ic0 operand lands just in time (first matmul ~9.5us).
Phase 2 streams w2 halves on sync+gpsimd; out rides the scalar queue.
Before the first real matmul, dummy N=128 matmuls on a memset scratch
tile warm the PE's HAM clock gate during the initial DMA wait, so real
matmuls start at 2.4 GHz instead of ramping from 1.2.
"""

import numpy as np
import ml_dtypes

T, A = 2048, 2
E, I, D = 8, 4096, 2048
N_CORES = 8
KC = D // 128   # 16 contraction chunks of 128 over D
IC = I // 128   # 32 i-chunks of 128
DC = D // 128   # 16 output d-chunks of 128

TRACE = False          # set by test harness to capture an NTFF profile
LAST_EXEC_NS = None    # filled when TRACE is set
_CACHE = {}            # compiled program cache keyed by C


def _build_program(C):
    import concourse.bass as bass
    import concourse.tile as tile
    from concourse import bacc, mybir

    f32 = mybir.dt.float32
    bf16 = mybir.dt.bfloat16

    nc = bacc.Bacc("TRN2", target_bir_lowering=False, debug=False,
                   num_devices=N_CORES)
    HK = KC // 2        # k-chunks in a weight low-half tile
    QK = KC // 4        # k-chunks in a weight quarter tile
    # x ships in 4 uniform chunks of 4 k-chunks, spread across queues
    x4_ap = nc.dram_tensor("x4", [4, 128, 4 * C], bf16, kind="ExternalInput").ap()
    w1_ap = nc.dram_tensor("w1", [IC, 128, KC * 128], bf16, kind="ExternalInput").ap()
    w3_ap = nc.dram_tensor("w3", [IC, 128, KC * 128], bf16, kind="ExternalInput").ap()
    w2_ap = nc.dram_tensor("w2", [DC, 128, IC * 128], bf16, kind="ExternalInput").ap()
    o_ap = nc.dram_tensor("o", [D, C], bf16, kind="ExternalOutput").ap()

    with tile.TileContext(nc) as tc:
        with tc.tile_pool(name="xpool", bufs=1) as xpool, \
             tc.tile_pool(name="hpool", bufs=1) as hpool, \
             tc.tile_pool(name="w13", bufs=6) as w13pool, \
             tc.tile_pool(name="w2p", bufs=2) as w2pool, \
             tc.tile_pool(name="act", bufs=2) as actpool, \
             tc.tile_pool(name="outp", bufs=2) as outpool, \
             tc.tile_pool(name="ps1", bufs=2, space="PSUM") as ps1, \
             tc.tile_pool(name="ps2", bufs=2, space="PSUM") as ps2:

            # PE pre-warm: dummy matmuls on a memset scratch tile keep the
            # HAM clock gate busy during the initial DMA wait
            scr = xpool.tile([128, 128], bf16, name="warm_scr")
            nc.gpsimd.memset(scr[:], 0)
            pdum = ps1.tile([128, 128], f32, tag="p1", name="pdum")
            for _ in range(36):
                nc.tensor.matmul(pdum[:], scr[:], scr[:], start=True, stop=True)
            scr2 = actpool.tile([128, 128], f32, tag="dumout", name="warm_out")
            nc.vector.tensor_copy(scr2[:], pdum[:])

            # ic0's weight tiles split in halves; x in four 0.5MB chunks,
            # queue order chosen so every ic0 operand lands just in time
            xts = [xpool.tile([128, 4 * C], bf16, name=f"xc{g}")
                   for g in range(4)]
            t1l0 = w13pool.tile([128, HK * 128], bf16, tag="tw1", name="tw1l_0")
            t1h0 = w13pool.tile([128, HK * 128], bf16, tag="tw1", name="tw1h_0")
            t3l0 = w13pool.tile([128, HK * 128], bf16, tag="tw3", name="tw3l_0")
            t3h0 = w13pool.tile([128, HK * 128], bf16, tag="tw3", name="tw3h_0")
            nc.scalar.dma_start(xts[0][:], x4_ap[0])        # kc 0-3
            nc.sync.dma_start(t1l0[:], w1_ap[0, :, :HK * 128])
            nc.gpsimd.dma_start(t3l0[:], w3_ap[0, :, :HK * 128])
            nc.scalar.dma_start(xts[1][:], x4_ap[1])        # kc 4-7
            nc.sync.dma_start(t1h0[:], w1_ap[0, :, HK * 128:])
            nc.gpsimd.dma_start(t3h0[:], w3_ap[0, :, HK * 128:])
            nc.scalar.dma_start(xts[2][:], x4_ap[2])        # kc 8-11
            nc.scalar.dma_start(xts[3][:], x4_ap[3])        # kc 12-15

            def xsl(kc):
                return xts[kc // 4][:, (kc % 4) * C:(kc % 4 + 1) * C]

            hts = [hpool.tile([128, C], bf16, name=f"ht_{ic}")
                   for ic in range(IC)]

            # ---- phase 1: hT = silu(w1T.T @ x) * (w3T.T @ x), per i-chunk ----
            for ic in range(IC):
                if ic == 0:
                    def wsl1(kc):
                        t = t1l0 if kc < HK else t1h0
                        return t[:, (kc % HK) * 128:(kc % HK + 1) * 128]

                    def wsl3(kc):
                        t = t3l0 if kc < HK else t3h0
                        return t[:, (kc % HK) * 128:(kc % HK + 1) * 128]
                else:
                    tw1 = w13pool.tile([128, KC * 128], bf16, tag="tw1",
                                       name=f"tw1_{ic}")
                    tw3 = w13pool.tile([128, KC * 128], bf16, tag="tw3",
                                       name=f"tw3_{ic}")
                    nc.sync.dma_start(tw1[:], w1_ap[ic])
                    nc.gpsimd.dma_start(tw3[:], w3_ap[ic])

                    def wsl1(kc, t=tw1):
                        return t[:, kc * 128:(kc + 1) * 128]

                    def wsl3(kc, t=tw3):
                        return t[:, kc * 128:(kc + 1) * 128]

                p1 = ps1.tile([128, C], f32, tag="p1", name=f"p1_{ic}")
                p3 = ps1.tile([128, C], f32, tag="p3", name=f"p3_{ic}")
                for kc in range(KC):
                    st, sp = (kc == 0), (kc == KC - 1)
                    nc.tensor.matmul(p1[:], wsl1(kc), xsl(kc), start=st, stop=sp)
                    nc.tensor.matmul(p3[:], wsl3(kc), xsl(kc), start=st, stop=sp)
                s1 = actpool.tile([128, C], f32, tag="s1", name=f"s1_{ic}")
                nc.scalar.activation(s1[:], p1[:],
                                     mybir.ActivationFunctionType.Silu)
                nc.vector.tensor_mul(hts[ic][:], s1[:], p3[:])

            # ---- phase 2: outT = w2T.T @ hT, per d-chunk ----
            for dc in range(DC):
                # stream w2 d-chunk in two halves on separate queues
                tw2a = w2pool.tile([128, (IC // 2) * 128], bf16, tag="tw2a",
                                   name=f"tw2a_{dc}")
                tw2b = w2pool.tile([128, (IC // 2) * 128], bf16, tag="tw2b",
                                   name=f"tw2b_{dc}")
                nc.sync.dma_start(tw2a[:], w2_ap[dc, :, :(IC // 2) * 128])
                nc.gpsimd.dma_start(tw2b[:], w2_ap[dc, :, (IC // 2) * 128:])
                ot = outpool.tile([128, C], bf16, tag="ot", name=f"ot_{dc}")
                # last d-chunk in two token-blocks so the final serial
                # copy+DMA tail after the last matmul is short
                blocks = [(0, C)] if dc < DC - 1 else [(0, C - 128), (C - 128, 128)]
                for bo, bn in blocks:
                    po = {}
                    for par in (0, 1):
                        po[par] = ps2.tile([128, bn], f32, tag=f"po_{par}",
                                           name=f"po_{dc}_{bo}_{par}")
                    for kic in range(IC):
                        half = tw2a if kic < IC // 2 else tw2b
                        j = kic % (IC // 2)
                        wsl = half[:, j * 128:(j + 1) * 128]
                        par = kic % 2
                        st, sp = (kic < 2), (kic >= IC - 2)
                        nc.tensor.matmul(po[par][:], wsl,
                                         hts[kic][:, bo:bo + bn],
                                         start=st, stop=sp)
                    osl = ot[:, bo:bo + bn]
                    nc.vector.tensor_copy(osl, po[0][:])
                    nc.vector.tensor_add(osl, osl, po[1][:])
                    nc.scalar.dma_start(
                        o_ap[dc * 128:(dc + 1) * 128, bo:bo + bn], osl)

    nc.compile()
    return nc


def _run_spmd(nc, in_maps):
    global LAST_EXEC_NS
    from concourse import bass_utils
    if TRACE:
        import sys, types
        try:
            from antenv.axon_hooks import get_axon_ntff_profile_hook  # noqa
        except ImportError:
            from trn_agent_boot.trn_boot import _ntff_profile_via_ctypes
            _hook = _ntff_profile_via_ctypes('/opt/axon/libaxon_pjrt.so')
            m = types.ModuleType("antenv.axon_hooks")
            m.get_axon_ntff_profile_hook = lambda: _hook
            sys.modules["antenv.axon_hooks"] = m
        bass_utils.upload_artifacts = lambda tmpdir: "local://" + tmpdir
    res = bass_utils.run_bass_kernel_spmd(
        nc, in_maps, core_ids=list(range(N_CORES)), trace=TRACE)
    if TRACE:
        LAST_EXEC_NS = res.exec_time_ns
    return res.results


def kernel(x, expert_indices, w1, w2, w3):
    x = np.asarray(x)
    ei = np.asarray(expert_indices)
    w1 = np.asarray(w1)
    w2 = np.asarray(w2)
    w3 = np.asarray(w3)

    # ---- host routing (dedup (token, expert) pairs) ----
    flat = ei.reshape(-1).astype(np.int64)          # assignment -> expert
    flat_tok = np.arange(T * A, dtype=np.int64) // A
    keys = flat * T + flat_tok                      # (expert, token) key
    uk = np.unique(keys)                            # sorted unique pairs
    ue, ut = uk // T, uk % T
    counts = np.bincount(ue, minlength=E)
    off = np.concatenate([[0], np.cumsum(counts)])
    C = int(counts.max())
    C += C % 2
    C = max(min(C, 512), 2)                         # cap: spill goes to host

    if C not in _CACHE:
        _CACHE[C] = _build_program(C)
    nc = _CACHE[C]

    # unique token row lists per expert (first C), padded with token 0
    tok = np.zeros((E, C), dtype=np.int64)
    ndev = np.minimum(counts, C)
    for e in range(E):
        tok[e, :ndev[e]] = ut[off[e]:off[e] + ndev[e]]

    bf = ml_dtypes.bfloat16
    w1b = w1.astype(bf)
    w2b = w2.astype(bf)
    w3b = w3.astype(bf)
    in_maps = []
    for e in range(E):
        xg = x[tok[e]]                                    # [C, D]
        # [KC, 128, C] -> 4 chunks of 4 k-chunks, each packed
        # [128, n*C] with k-chunk-major columns
        xT = xg.T.astype(bf).reshape(KC, 128, C)

        def pack(lo, n):
            return np.ascontiguousarray(
                xT[lo:lo + n].transpose(1, 0, 2)).reshape(128, n * C)
        x4 = np.stack([pack(4 * g, 4) for g in range(4)])  # [4, 128, 4C]
        # w1/w3 [I, D] -> [ic, j, kc, p] -> [ic, p, kc, j]
        w1p = np.ascontiguousarray(
            w1b[e].reshape(IC, 128, KC, 128).transpose(0, 3, 2, 1)
        ).reshape(IC, 128, KC * 128)
        w3p = np.ascontiguousarray(
            w3b[e].reshape(IC, 128, KC, 128).transpose(0, 3, 2, 1)
        ).reshape(IC, 128, KC * 128)
        # w2 [D, I] -> [dc, j, kic, p] -> [dc, p, kic, j]
        w2p = np.ascontiguousarray(
            w2b[e].reshape(DC, 128, IC, 128).transpose(0, 3, 2, 1)
        ).reshape(DC, 128, IC * 128)
        in_maps.append({"x4": x4, "w1": w1p, "w3": w3p, "w2": w2p})

    results = _run_spmd(nc, in_maps)

    # ---- host scatter (each assignment gathers its expert's row) ----
    R = np.stack([np.asarray(results[e]["o"]).astype(np.float32)
                  for e in range(E)])                 # [E, D, C]
    pos = np.searchsorted(uk, keys)                   # row within expert block
    row = pos - off[flat]
    on_dev = row < ndev[flat]
    out_flat = np.empty((T * A, D), dtype=np.float32)
    out_flat[on_dev] = R[flat[on_dev], :, row[on_dev]]

    # spill (unique pairs beyond C per expert): host compute
    if not np.all(on_dev):
        sp = np.nonzero(~on_dev)[0]
        for i in sp:
            e, t = flat[i], flat_tok[i]
            xs = x[t]
            h1 = xs @ w1[e].T
            h3 = xs @ w3[e].T
            h = (h1 / (1.0 + np.exp(-h1))) * h3
            out_flat[i] = h @ w2[e].T
    return out_flat.reshape(T, A, D)
